# revision 13
# baseline (speedup 1.0000x reference)
"""Trainium2 Bass kernel for nn_CCL_Module (3x3 cost-volume softmax flow).

Reference computation (per batch):
  c1 = l2norm_C(feature1); wp = l2norm_C(feature2) zero-padded spatially.
  match_vol[d=(dh,dw)] = sum_C c1 * shift(wp, dh, dw)      (9 shifts, 3x3)
  p = softmax(10 * match_vol, over d)
  flow_w = sum_d p * dw ; flow_h = sum_d p * dh
  out = concat([flow_w, flow_h])  -> [B, 2, H, W]

Strategy (pure data parallel, one batch per NeuronCore, 8 cores):
  - SBUF layout: partitions = (hh, c) with hh the H-half (2) and c the
    channel (64); free dim = (h', w) flat (64*128 = 8192).  Loads from
    DRAM are 128 contiguous 32KB descriptors with SWDGE f32->fp16 cast.
  - All 9 (dh, dw) shifts are free-dim offsets delta = 128*dh + dw into a
    zero-padded copy of feature2.  A second copy shifted by one element
    keeps odd-delta operands 4-byte aligned so DVE fp16 muls run at 2x.
  - Channel reduction via TensorE: matmul with a sliding one-hot
    column-pair mask [128, 128] (ones over partitions 0-63 in col 2k,
    over 64-127 in col 2k+1) reduces chunk k of a product over c and
    drops row sums into PSUM rows (2k, 2k+1); 64 accumulating matmuls
    per map fill a [128, 128] fp32 score tile laid out as
    [p = 2h'+hh, w].  11 maps: 9 correlations + |f1|^2 + |f2|^2.
  - L2 normalization folded into score scaling:
      score_d = 10 * A_d * rsqrt(|f1|^2) * rsqrt(|f2|^2 shifted)
    Scores bounded by 10 so softmax needs no max subtraction.
"""

import numpy as np

B, C, H, W = 8, 64, 128, 128
N_CORES = 8
SOFTMAX_SCALE = 10.0

HH = 2
HP = H // HH            # 64 h-rows per half
FREE = HP * W           # 8192 free elems per partition
M0 = 160                # main-data column offset in padded f2 tiles
F2W = M0 + FREE + 160   # 8512

_CACHE = {}


def _build_program():
    import concourse.bass as bass
    import concourse.bacc as bacc
    import concourse.mybir as mybir
    from concourse.tile import TileContext
    from concourse.bass_utils import axon_active

    f32 = mybir.dt.float32
    f16 = mybir.dt.float16
    AF = mybir.ActivationFunctionType
    red = dict(axis=mybir.AxisListType.X, op=mybir.AluOpType.add)

    nc = bacc.Bacc(
        "TRN2",
        target_bir_lowering=False,
        debug=not axon_active(),
        num_devices=N_CORES,
    )

    f1d = nc.declare_dram_parameter("feature1", [C, H, W], f32, isOutput=False)
    f2d = nc.declare_dram_parameter("feature2", [C, H, W], f32, isOutput=False)
    outd = nc.declare_dram_parameter("flow", [2, H, W], f32, isOutput=True)

    # [64, 8192] per-half views: partition c, free = h'*W + w
    f1h = [
        f1d[:, hh * HP : (hh + 1) * HP, :].rearrange("c hp w -> c (hp w)")
        for hh in range(HH)
    ]
    f2h = [
        f2d[:, hh * HP : (hh + 1) * HP, :].rearrange("c hp w -> c (hp w)")
        for hh in range(HH)
    ]
    # output view: partition = h (score layout), free = (o, w)
    outv = outd.rearrange("o h w -> h o w")

    with TileContext(nc) as tc:
        with tc.tile_pool(name="main", bufs=1) as pool, \
             tc.tile_pool(name="prod", bufs=14) as prodp, \
             tc.tile_pool(name="psum", bufs=1, space="PSUM") as psp:

            # ---- input tiles (fp16, cast during SWDGE DMA) ----
            xf1 = pool.tile([128, FREE], f16)
            xf2 = pool.tile([128, F2W], f16)    # even-parity padded f2
            xf2o = pool.tile([128, F2W], f16)   # same, shifted 1 elem left

            nc.gpsimd.dma_start(out=xf1[0:64, :], in_=f1h[0])
            nc.gpsimd.dma_start(out=xf1[64:128, :], in_=f1h[1])

            # zero the pad regions (h'=-1 / h'=64 halos + spare)
            nc.vector.memset(xf2[:, 0:M0], 0.0)
            nc.vector.memset(xf2[:, M0 + FREE : F2W], 0.0)
            nc.vector.memset(xf2o[:, 0 : M0 - 1], 0.0)
            nc.vector.memset(xf2o[:, M0 - 1 + FREE : F2W], 0.0)

            # main f2 data
            nc.gpsimd.dma_start(out=xf2[0:64, M0 : M0 + FREE], in_=f2h[0])
            nc.gpsimd.dma_start(out=xf2[64:128, M0 : M0 + FREE], in_=f2h[1])
            nc.sync.dma_start(
                out=xf2o[0:64, M0 - 1 : M0 - 1 + FREE],
                in_=xf2[0:64, M0 : M0 + FREE],
            )
            nc.sync.dma_start(
                out=xf2o[64:128, M0 - 1 : M0 - 1 + FREE],
                in_=xf2[64:128, M0 : M0 + FREE],
            )
            # halo rows: hh=0 needs h=64 above its top edge is h=-1 (zero),
            # below its last row h'=63 is h=64; hh=1 has h=63 above, zero below.
            nc.gpsimd.dma_start(
                out=xf2[64:128, M0 - 128 : M0], in_=f2d[:, HP - 1, :]
            )
            nc.gpsimd.dma_start(
                out=xf2[0:64, M0 + FREE : M0 + FREE + 128], in_=f2d[:, HP, :]
            )
            nc.sync.dma_start(
                out=xf2o[64:128, M0 - 129 : M0 - 1],
                in_=xf2[64:128, M0 - 128 : M0],
            )
            nc.sync.dma_start(
                out=xf2o[0:64, M0 - 1 + FREE : M0 - 1 + FREE + 128],
                in_=xf2[0:64, M0 + FREE : M0 + FREE + 128],
            )

            # ---- sliding one-hot mask for the channel-reduce matmuls ----
            # S_k = smask[:, 63-k : 191-k]:
            #   S_k[p, k]    = 1 for p in [0,64)   -> PSUM row k    = h = k
            #   S_k[p, 64+k] = 1 for p in [64,128) -> row 64+k      = h = 64+k
            smask = pool.tile([128, 191], f16)
            nc.vector.memset(smask[:, :], 0.0)
            nc.vector.memset(smask[0:64, 63:64], 1.0)
            nc.vector.memset(smask[64:128, 127:128], 1.0)

            # ---- PSUM score tiles: one full bank per concurrently
            # accumulating map (start=True clears has_written bank-wide,
            # so interleaved accumulation groups must not share a bank).
            ptiles = [
                psp.tile([128, 128], f32, tag=f"pb{i}", name=f"pb{i}")
                for i in range(8)
            ]
            # s_all score columns: d*128 for d=0..8, |f1|^2 @1152, |f2|^2 @1280
            s_all = pool.tile([128, 1408], f32)
            SCOL = {d: d * 128 for d in range(9)}
            SCOL[9], SCOL[10] = 1152, 1280
            sc = s_all[:, 0:1152]

            def shift_view(dh, dw):
                delta = 128 * dh + dw
                if dw == 0:
                    return xf2[:, M0 + delta : M0 + delta + FREE]
                return xf2o[:, M0 + delta - 1 : M0 + delta - 1 + FREE]

            groups = [[9], [10, 4, 1], [7, 3, 5, 0], [2, 6, 8]]
            gbanks = [ptiles[0:1], ptiles[1:4], ptiles[4:8], ptiles[0:3]]

            # tail tiles (ops emitted inside the group loop below)
            r1 = pool.tile([128, 128], f32)
            r2p = pool.tile([128, 130], f32)
            r2hp = pool.tile([128, 130], f32)
            r2hm = pool.tile([128, 130], f32)
            expo = pool.tile([128, 1152], f32)

            def normalize_map(d):
                # shat_d = A_d * r1 * r2[h+dh, w+dw]; w-edge wraps are 0 in
                # the reference, then e_d = exp(10 * shat_d)
                dh, dw = d // 3 - 1, d % 3 - 1
                r2x = (r2hm, r2p, r2hp)[dh + 1]
                sd = sc[:, d * 128 : (d + 1) * 128]
                nc.vector.tensor_mul(sd, sd, r2x[:, 1 + dw : 129 + dw])
                nc.vector.tensor_mul(sd, sd, r1[:, :])
                if dw == -1:
                    nc.vector.memset(sd[:, 0:1], 0.0)
                elif dw == 1:
                    nc.vector.memset(sd[:, 127:128], 0.0)
                nc.scalar.activation(
                    expo[:, d * 128 : (d + 1) * 128],
                    sd,
                    AF.Exp,
                    scale=SOFTMAX_SCALE,
                )

            HALF = FREE // 2
            for gi, g in enumerate(groups):
                banks = gbanks[gi]
                prs = []
                for m in g:
                    # half-size product tiles: matmuls for chunks k<32 only
                    # need the first half, so the PE unblocks sooner and
                    # pool slots recycle at twice the rate
                    ph = []
                    for hv in range(2):
                        pr = prodp.tile([128, HALF], f16, tag="prod",
                                        name=f"pr{m}_{hv}")
                        lo = HALF * hv
                        if m == 9:
                            CH = HALF // 2
                            for ci in range(2):
                                cs = slice(lo + CH * ci, lo + CH * (ci + 1))
                                co = slice(CH * ci, CH * (ci + 1))
                                nc.vector.tensor_mul(
                                    pr[:, co], xf1[:, cs], xf1[:, cs]
                                )
                        elif m == 10:
                            nc.scalar.activation(
                                pr[:, :],
                                xf2[:, M0 + lo : M0 + lo + HALF],
                                AF.Square,
                            )
                        else:
                            dh, dw = m // 3 - 1, m % 3 - 1
                            nc.vector.tensor_mul(
                                pr[:, :],
                                xf1[:, lo : lo + HALF],
                                shift_view(dh, dw)[:, lo : lo + HALF],
                            )
                        ph.append(pr)
                    prs.append(ph)
                for k in range(HP):
                    lhs = smask[:, 63 - k : 191 - k]
                    hv, kk = divmod(k, 32)
                    for ph, bank in zip(prs, banks):
                        nc.tensor.matmul(
                            bank[:, :],
                            lhs,
                            ph[hv][:, 128 * kk : 128 * (kk + 1)],
                            start=(k == 0),
                            stop=(k == HP - 1),
                        )
                # drain this group's scores to SBUF, then normalize + exp the
                # finished correlation maps while later groups keep the PE busy
                for m, bank in zip(g, banks):
                    nc.scalar.copy(
                        s_all[:, SCOL[m] : SCOL[m] + 128], bank[:, :]
                    )
                if gi == 0:
                    # |f1|^2 done -> r1 = rsqrt(n1)
                    nc.scalar.sqrt(r1[:, :], s_all[:, 1152:1280])
                    nc.vector.reciprocal(r1[:, :], r1[:, :])
                elif gi == 1:
                    # |f2|^2 done -> r2 = rsqrt(n2), padded + h-shifted copies
                    nc.vector.memset(r2p[:, 0:1], 1.0)
                    nc.vector.memset(r2p[:, 129:130], 1.0)
                    nc.scalar.sqrt(r2p[:, 1:129], s_all[:, 1280:1408])
                    nc.vector.reciprocal(r2p[:, 1:129], r2p[:, 1:129])
                    nc.sync.dma_start(out=r2hp[0:127, :], in_=r2p[1:128, :])
                    nc.sync.dma_start(
                        out=r2hp[127:128, :], in_=r2p[127:128, :]
                    )
                    nc.sync.dma_start(out=r2hm[1:128, :], in_=r2p[0:127, :])
                    nc.sync.dma_start(out=r2hm[0:1, :], in_=r2p[0:1, :])
                    for m in g:
                        if m < 9:
                            normalize_map(m)
                else:
                    for m in g:
                        normalize_map(m)

            # ---- softmax-weighted displacement sums ----
            esum = pool.tile([128, 128], f32)
            fwp = pool.tile([128, 128], f32)
            fwm = pool.tile([128, 128], f32)
            fhp = pool.tile([128, 128], f32)
            fhm = pool.tile([128, 128], f32)
            ex4 = expo.rearrange("p (a b w) -> p a b w", a=3, b=3)
            nc.vector.tensor_reduce(
                esum[:, :], expo.rearrange("p (d w) -> p w d", d=9), **red
            )
            nc.vector.tensor_reduce(
                fwp[:, :], ex4[:, :, 2, :].rearrange("p a w -> p w a"), **red
            )
            nc.vector.tensor_reduce(
                fwm[:, :], ex4[:, :, 0, :].rearrange("p a w -> p w a"), **red
            )
            nc.vector.tensor_reduce(
                fhp[:, :], ex4[:, 2, :, :].rearrange("p b w -> p w b"), **red
            )
            nc.vector.tensor_reduce(
                fhm[:, :], ex4[:, 0, :, :].rearrange("p b w -> p w b"), **red
            )

            flows = pool.tile([128, 2, W], f32)
            nc.vector.reciprocal(esum[:, :], esum[:, :])
            nc.vector.tensor_sub(fwp[:, :], fwp[:, :], fwm[:, :])
            nc.vector.tensor_sub(fhp[:, :], fhp[:, :], fhm[:, :])
            nc.vector.tensor_mul(flows[:, 0, :], fwp[:, :], esum[:, :])
            nc.vector.tensor_mul(flows[:, 1, :], fhp[:, :], esum[:, :])

            nc.sync.dma_start(out=outv, in_=flows[:, :, :])

    nc.compile()
    return nc


LAST_RESULT = None


def kernel(feature1: np.ndarray, feature2: np.ndarray) -> np.ndarray:
    global LAST_RESULT
    from concourse import bass_utils

    if "nc" not in _CACHE:
        _CACHE["nc"] = _build_program()
    nc = _CACHE["nc"]

    f1 = np.ascontiguousarray(np.asarray(feature1, dtype=np.float32))
    f2 = np.ascontiguousarray(np.asarray(feature2, dtype=np.float32))
    in_maps = [
        {"feature1": f1[b], "feature2": f2[b]} for b in range(N_CORES)
    ]
    res = bass_utils.run_bass_kernel_spmd(nc, in_maps, list(range(N_CORES)))
    LAST_RESULT = res
    out = np.stack([res.results[b]["flow"] for b in range(N_CORES)], axis=0)
    return out.astype(np.float32)
